# revision 2
# baseline (speedup 1.0000x reference)
"""Binarized conv2d kernel for Trainium2, SPMD over 8 NeuronCores.

Math (forward-value equivalent of the reference):
    real_w  = sum_k RV[k] * weights[k]          # [256,256,3,3], exact fp32 on DVE
    scale   = mean(|real_w|, axis=(1,2,3))      # per out-channel
    out     = conv2d(sign(x), sign(real_w), pad=1) * (scale * alpha)

sign(x) and sign(real_w) are {-1,0,+1} which are exact in fp8e4, so the conv
is computed with fp8 DoubleRow matmuls (exact integer accumulation in fp32
PSUM) and the per-channel scale*alpha is applied on PSUM evacuation.

Sharding: data-parallel over batch, 4 images per core; weights/RV/alpha
replicated. No collectives.
"""

import numpy as np
from contextlib import ExitStack

import concourse.bass as bass
import concourse.bacc as bacc
import concourse.tile as tile
from concourse import mybir
from concourse.bass_utils import run_bass_kernel_spmd
from concourse.masks import make_identity

# Problem shapes (hardcoded per contract)
B, C, H, W = 32, 256, 56, 56
K, KS = 4, 3
NCORES = 8
BL = B // NCORES            # images per core

PW = W + 2                  # padded width 58
PLANE = PW * PW             # 3364
PL = 3376                   # plane stride (>= 1+PLANE+1, multiple of 16)
GO = 1                      # guard offset: plane data starts at elem 1
RPC = 7                     # rows per chunk
CHUNK = RPC * PW            # 406 elems per matmul (one PSUM bank)
CPT = 4                     # chunks per psum tile
PTS = 2                     # psum tiles per (img, co-half) -> 2*4*7 = 56 rows
CIH = C // 128              # 2 ci halves
COH = C // 128              # 2 co halves
TAPS = KS * KS              # 9

F32 = mybir.dt.float32
FP8 = mybir.dt.float8e4
BF16 = mybir.dt.bfloat16

USE_DR = True               # fp8 DoubleRow (2x matmul) vs bf16

_cache = {}


def _build():
    act_dt = FP8 if USE_DR else BF16
    nc = bacc.Bacc("TRN2", target_bir_lowering=False, debug=False,
                   num_devices=NCORES)
    x_d = nc.dram_tensor("x", [BL, C, H, W], F32, kind="ExternalInput")
    w_d = nc.dram_tensor("weights", [K, C, C, KS, KS], F32, kind="ExternalInput")
    rv_d = nc.dram_tensor("RV", [K + 1], F32, kind="ExternalInput")
    al_d = nc.dram_tensor("alpha", [C, 1, 1], F32, kind="ExternalInput")
    o_d = nc.dram_tensor("out", [BL, C, H, W], F32, kind="ExternalOutput")

    with tile.TileContext(nc) as tc, ExitStack() as ctx:
        consts = ctx.enter_context(tc.tile_pool(name="consts", bufs=1))
        wstage = ctx.enter_context(tc.tile_pool(name="wstage", bufs=5))
        wwork = ctx.enter_context(tc.tile_pool(name="wwork", bufs=2))
        xin = ctx.enter_context(tc.tile_pool(name="xin", bufs=3))
        xpads = ctx.enter_context(tc.tile_pool(name="xpads", bufs=1))
        outp = ctx.enter_context(tc.tile_pool(name="outp", bufs=3))

        # --- constants -----------------------------------------------------
        ident = consts.tile([128, 128], FP8 if USE_DR else BF16, tag="ident")
        make_identity(nc, ident)

        rv1 = consts.tile([1, K], F32, tag="rv1")
        nc.sync.dma_start(out=rv1, in_=rv_d.ap()[0:K].rearrange("(a b) -> a b", a=1))
        rv = consts.tile([128, K], F32, tag="rv")
        nc.gpsimd.partition_broadcast(rv, rv1)

        alpha_sb = []
        for h in range(COH):
            t = consts.tile([128, 1], F32, tag=f"alpha{h}")
            nc.sync.dma_start(out=t, in_=al_d.ap()[h * 128:(h + 1) * 128, 0, :])
            alpha_sb.append(t)

        # --- weight prep: mix, scale, sign ---------------------------------
        wsign = []
        scale_alpha = []
        for h in range(COH):
            wmix = wwork.tile([128, C * TAPS], F32, tag="wmix")
            for k in range(K):
                wk = wstage.tile([128, C * TAPS], F32, tag="wsb")
                nc.sync.dma_start(
                    out=wk,
                    in_=w_d.ap()[k, h * 128:(h + 1) * 128].rearrange(
                        "p c a b -> p (c a b)"))
                if k == 0:
                    nc.vector.tensor_scalar(wmix, wk, rv[:, 0:1], None,
                                            mybir.AluOpType.mult)
                else:
                    nc.vector.scalar_tensor_tensor(
                        wmix, wk, rv[:, k:k + 1], wmix,
                        mybir.AluOpType.mult, mybir.AluOpType.add)
            absum = consts.tile([128, 1], F32, tag=f"absum{h}")
            nc.vector.tensor_reduce(absum, wmix, mybir.AxisListType.X,
                                    mybir.AluOpType.add,
                                    apply_absolute_value=True)
            sa = consts.tile([128, 1], F32, tag=f"sa{h}")
            nc.vector.scalar_tensor_tensor(
                sa, absum, 1.0 / (C * TAPS), alpha_sb[h],
                mybir.AluOpType.mult, mybir.AluOpType.mult)
            scale_alpha.append(sa)
            ws = wwork.tile([128, C * TAPS], act_dt, tag=f"wsign{h}", bufs=1)
            nc.scalar.sign(ws, wmix)
            wsign.append(ws)

        # --- transpose weights to [ci, (tap, coh, cih), co] ----------------
        wT = consts.tile([128, TAPS, COH, CIH, 128], act_dt, tag="wT")
        with tc.tile_pool(name="tpsum", bufs=4, space="PSUM") as tpsum:
            for tap in range(TAPS):
                for h in range(COH):
                    wsv = wsign[h].rearrange("p (ci t) -> p ci t", t=TAPS)
                    for ci in range(CIH):
                        tp = tpsum.tile([128, 128], F32, tag="tp")
                        nc.tensor.matmul(
                            tp, wsv[:, ci * 128:(ci + 1) * 128, tap], ident,
                            start=True, stop=True)
                        nc.vector.tensor_copy(wT[:, tap, h, ci, :], tp)

        # --- padded sign(x) planes (memset once; pads never rewritten) -----
        xpad = []
        for i in range(2):
            t = xpads.tile([128, CIH, PL], act_dt, tag=f"xpad{i}")
            nc.gpsimd.memset(t, 0.0)
            xpad.append(t)

        # --- main loop over images -----------------------------------------
        with tc.tile_pool(name="cpsum", bufs=2, space="PSUM") as cpsum:
            for b in range(BL):
                xp = xpad[b % 2]
                for s in range(CIH):
                    xs = xin.tile([128, H * W], F32, tag="xsb")
                    nc.sync.dma_start(
                        out=xs, in_=x_d.ap()[b, s * 128:(s + 1) * 128].rearrange(
                            "p a b -> p (a b)"))
                    dst = xp[:, s, GO:GO + PLANE].rearrange(
                        "p (y x) -> p y x", x=PW)[:, 1:57, 1:57]
                    nc.scalar.sign(dst, xs.rearrange("p (y x) -> p y x", x=W))

                for h in range(COH):
                    osb = outp.tile([128, H * W], F32, tag="osb")
                    for t in range(PTS):
                        ps = cpsum.tile([128, CPT * 512], F32, tag="ps")
                        for tap in range(TAPS):
                            dy, dx = tap // KS - 1, tap % KS - 1
                            if USE_DR:
                                lhsT = wT[:, tap, h, :, :]
                            for j in range(CPT):
                                c = t * CPT + j
                                off = GO + (1 + RPC * c + dy) * PW + dx
                                o = ps[:, j * 512:j * 512 + CHUNK]
                                if USE_DR:
                                    nc.tensor.matmul(
                                        o, lhsT, xp[:, :, off:off + CHUNK],
                                        start=(tap == 0), stop=(tap == TAPS - 1),
                                        perf_mode=mybir.MatmulPerfMode.DoubleRow)
                                else:
                                    for s in range(CIH):
                                        nc.tensor.matmul(
                                            o, wT[:, tap, h, s, :],
                                            xp[:, s, off:off + CHUNK],
                                            start=(tap == 0 and s == 0),
                                            stop=(tap == TAPS - 1 and s == CIH - 1))
                        src = ps.rearrange("p (c e) -> p c e", e=512)[
                            :, :, 0:CHUNK].rearrange(
                            "p c (r x) -> p c r x", x=PW)[:, :, :, 1:57]
                        dst = osb.rearrange("p (y x) -> p y x", x=W)[
                            :, t * CPT * RPC:(t + 1) * CPT * RPC, :].rearrange(
                            "p (c r) x -> p c r x", r=RPC)
                        nc.scalar.activation(dst, src,
                                             mybir.ActivationFunctionType.Copy,
                                             bias=0.0, scale=scale_alpha[h])
                    nc.sync.dma_start(
                        out=o_d.ap()[b, h * 128:(h + 1) * 128].rearrange(
                            "p a b -> p (a b)"),
                        in_=osb)
    nc.compile()
    return nc


def _get_nc():
    if "nc" not in _cache:
        _cache["nc"] = _build()
    return _cache["nc"]


def run(inputs, trace=False):
    nc = _get_nc()
    x = np.ascontiguousarray(inputs["x"], dtype=np.float32)
    in_maps = [
        {
            "x": x[c * BL:(c + 1) * BL],
            "weights": np.ascontiguousarray(inputs["weights"], np.float32),
            "RV": np.ascontiguousarray(inputs["RV"], np.float32),
            "alpha": np.ascontiguousarray(inputs["alpha"], np.float32),
        }
        for c in range(NCORES)
    ]
    res = run_bass_kernel_spmd(nc, in_maps, core_ids=list(range(NCORES)),
                               trace=trace)
    out = np.concatenate([r["out"] for r in res.results], axis=0)
    return out, res


def kernel(**inputs) -> np.ndarray:
    out, _ = run(inputs, trace=False)
    return out


# revision 6
# speedup vs baseline: 1.0643x; 1.0643x over previous
"""Binarized conv2d kernel for Trainium2, SPMD over 8 NeuronCores.

Math (forward-value equivalent of the reference):
    real_w  = sum_k RV[k] * weights[k]          # [256,256,3,3], exact fp32 on DVE
    scale   = mean(|real_w|, axis=(1,2,3))      # per out-channel
    out     = conv2d(sign(x), sign(real_w), pad=1) * (scale * alpha)

sign(x) and sign(real_w) are {-1,0,+1} which are exact in fp8e4, so the conv
is computed with fp8 DoubleRow matmuls (exact integer accumulation in fp32
PSUM) and the per-channel scale*alpha is applied on PSUM evacuation.

Sharding: data-parallel over batch, 4 images per core; weights/RV/alpha
replicated. No collectives.
"""

import numpy as np
from contextlib import ExitStack

import concourse.bass as bass
import concourse.bacc as bacc
import concourse.tile as tile
from concourse import mybir
from concourse.bass_utils import run_bass_kernel_spmd
from concourse.masks import make_identity

# Problem shapes (hardcoded per contract)
B, C, H, W = 32, 256, 56, 56
K, KS = 4, 3
NCORES = 8
BL = B // NCORES            # images per core

PW = W + 2                  # padded width 58
PLANE = PW * PW             # 3364
PL = 3376                   # plane stride (>= 1+PLANE+1, multiple of 16)
GO = 1                      # guard offset: plane data starts at elem 1
RPC = 7                     # rows per chunk
CHUNK = RPC * PW            # 406 elems per matmul (one PSUM bank)
CPT = 4                     # chunks per psum tile
PTS = 2                     # psum tiles per (img, co-half) -> 2*4*7 = 56 rows
CIH = C // 128              # 2 ci halves
COH = C // 128              # 2 co halves
TAPS = KS * KS              # 9

F32 = mybir.dt.float32
FP8 = mybir.dt.float8e4
BF16 = mybir.dt.bfloat16

USE_DR = True               # fp8 DoubleRow (2x matmul) vs bf16

_cache = {}


def _build():
    act_dt = FP8 if USE_DR else BF16
    nc = bacc.Bacc("TRN2", target_bir_lowering=False, debug=False,
                   num_devices=NCORES)
    x_d = nc.dram_tensor("x", [BL, C, H, W], F32, kind="ExternalInput")
    w_d = nc.dram_tensor("weights", [K, C, C, KS, KS], F32, kind="ExternalInput")
    rv_d = nc.dram_tensor("RV", [K + 1], F32, kind="ExternalInput")
    al_d = nc.dram_tensor("alpha", [C, 1, 1], F32, kind="ExternalInput")
    o_d = nc.dram_tensor("out", [BL, C, H, W], F32, kind="ExternalOutput")

    with tile.TileContext(nc) as tc, ExitStack() as ctx:
        consts = ctx.enter_context(tc.tile_pool(name="consts", bufs=1))
        wstage = ctx.enter_context(tc.tile_pool(name="wstage", bufs=5))
        wwork = ctx.enter_context(tc.tile_pool(name="wwork", bufs=2))
        xin = ctx.enter_context(tc.tile_pool(name="xin", bufs=3))
        xpads = ctx.enter_context(tc.tile_pool(name="xpads", bufs=1))
        outp = ctx.enter_context(tc.tile_pool(name="outp", bufs=3))

        # --- tiny constant loads (keep the HWDGE ring front clear) ---------
        rv = consts.tile([128, K], F32, tag="rv")
        rv_src = bass.AP(tensor=rv_d.ap().tensor, offset=0,
                         ap=[[0, 128], [1, K]])
        nc.sync.dma_start(out=rv, in_=rv_src)
        alpha_sb = []
        for h in range(COH):
            t = consts.tile([128, 1], F32, tag=f"alpha{h}")
            nc.sync.dma_start(out=t, in_=al_d.ap()[h * 128:(h + 1) * 128, 0, :])
            alpha_sb.append(t)

        # gpsimd-side constants: padded-plane memsets + identity, all early
        xpad = []
        for i in range(2):
            t = xpads.tile([128, CIH, PL], act_dt, tag=f"xpad{i}")
            nc.gpsimd.memset(t, 0.0)
            xpad.append(t)
        ident = consts.tile([128, 128], act_dt, tag="ident")
        make_identity(nc, ident)

        wT = consts.tile([128, TAPS, COH, CIH, 128], act_dt, tag="wT")
        scale_alpha = [consts.tile([128, 1], F32, tag=f"sa{h}", name=f"sa{h}")
                       for h in range(COH)]

        # --- weight prep for one co-half: DMA, mix, scale, sign ------------
        def prep_half(h):
            wmix = wwork.tile([128, C * TAPS], F32, tag="wmix")
            for k in range(K):
                wk = wstage.tile([128, C * TAPS], F32, tag="wsb")
                nc.sync.dma_start(
                    out=wk,
                    in_=w_d.ap()[k, h * 128:(h + 1) * 128].rearrange(
                        "p c a b -> p (c a b)"))
                if k == 0:
                    nc.vector.tensor_scalar(wmix, wk, rv[:, 0:1], None,
                                            mybir.AluOpType.mult)
                else:
                    nc.vector.scalar_tensor_tensor(
                        wmix, wk, rv[:, k:k + 1], wmix,
                        mybir.AluOpType.mult, mybir.AluOpType.add)
            ws = wwork.tile([128, C * TAPS], act_dt, tag=f"wsign{h}", bufs=1)
            nc.scalar.sign(ws, wmix)
            absum = consts.tile([128, 1], F32, tag=f"absum{h}")
            nc.vector.tensor_reduce(absum, wmix, mybir.AxisListType.X,
                                    mybir.AluOpType.add,
                                    apply_absolute_value=True)
            nc.vector.scalar_tensor_tensor(
                scale_alpha[h], absum, 1.0 / (C * TAPS), alpha_sb[h],
                mybir.AluOpType.mult, mybir.AluOpType.mult)
            return ws

        # --- transpose one co-half's sign-weights into wT -------------------
        # Both ci-half transposes of a tap land in bank 0 of a conv-pool psum
        # tile ([128,128] f32 at offsets 0 and 128 both fit in one bank) and
        # are evacuated with a single DVE copy.
        def transpose_half(h, wsgn, cpsum):
            wsv = wsgn.rearrange("p (ci t) -> p ci t", t=TAPS)
            for tap in range(TAPS):
                tp = cpsum.tile([128, CPT * 512], F32, tag="ps")
                for ci in range(CIH):
                    nc.tensor.matmul(
                        tp[:, ci * 128:(ci + 1) * 128],
                        wsv[:, ci * 128:(ci + 1) * 128, tap], ident,
                        start=True, stop=True)
                nc.vector.tensor_copy(
                    wT[:, tap, h, :, :],
                    tp[:, 0:CIH * 128].rearrange("p (ci co) -> p ci co", co=128))

        # --- load + sign one image into its padded plane --------------------
        def load_sign(b):
            xp = xpad[b % 2]
            for s in range(CIH):
                xs = xin.tile([128, H * W], F32, tag="xsb")
                nc.sync.dma_start(
                    out=xs, in_=x_d.ap()[b, s * 128:(s + 1) * 128].rearrange(
                        "p a b -> p (a b)"))
                dst = xp[:, s, GO:GO + PLANE].rearrange(
                    "p (y x) -> p y x", x=PW)[:, 1:57, 1:57]
                nc.scalar.sign(dst, xs.rearrange("p (y x) -> p y x", x=W))

        # --- conv for one (image, co-half) ---------------------------------
        def conv(b, h, cpsum):
            xp = xpad[b % 2]
            osb = outp.tile([128, H * W], F32, tag="osb")
            for t in range(PTS):
                ps = cpsum.tile([128, CPT * 512], F32, tag="ps")
                for tap in range(TAPS):
                    dy, dx = tap // KS - 1, tap % KS - 1
                    lhsT = wT[:, tap, h, :, :]
                    for j in range(CPT):
                        c = t * CPT + j
                        off = GO + (1 + RPC * c + dy) * PW + dx
                        o = ps[:, j * 512:j * 512 + CHUNK]
                        if USE_DR:
                            nc.tensor.matmul(
                                o, lhsT, xp[:, :, off:off + CHUNK],
                                start=(tap == 0), stop=(tap == TAPS - 1),
                                perf_mode=mybir.MatmulPerfMode.DoubleRow)
                        else:
                            for s in range(CIH):
                                nc.tensor.matmul(
                                    o, wT[:, tap, h, s, :],
                                    xp[:, s, off:off + CHUNK],
                                    start=(tap == 0 and s == 0),
                                    stop=(tap == TAPS - 1 and s == CIH - 1))
                src = ps.rearrange("p (c e) -> p c e", e=512)[
                    :, :, 0:CHUNK].rearrange(
                    "p c (r x) -> p c r x", x=PW)[:, :, :, 1:57]
                dst = osb.rearrange("p (y x) -> p y x", x=W)[
                    :, t * CPT * RPC:(t + 1) * CPT * RPC, :].rearrange(
                    "p (c r) x -> p c r x", r=RPC)
                # balance PSUM evacuation: co-half 0 on DVE, co-half 1 on ACT
                if h == 0:
                    nc.vector.tensor_scalar(dst, src, scale_alpha[h], None,
                                            mybir.AluOpType.mult)
                else:
                    nc.scalar.activation(dst, src,
                                         mybir.ActivationFunctionType.Copy,
                                         bias=0.0, scale=scale_alpha[h])
            nc.sync.dma_start(
                out=o_d.ap()[b, h * 128:(h + 1) * 128].rearrange(
                    "p a b -> p (a b)"),
                in_=osb)

        # --- schedule --------------------------------------------------------
        ws0 = prep_half(0)
        load_sign(0)                   # x[0] DMA right behind w-half-0 DMAs
        ws1 = prep_half(1)
        load_sign(1)

        with tc.tile_pool(name="cpsum", bufs=2, space="PSUM") as cpsum:
            transpose_half(0, ws0, cpsum)
            conv(0, 0, cpsum)
            transpose_half(1, ws1, cpsum)
            conv(0, 1, cpsum)
            for b in range(1, BL):
                if b + 1 < BL:
                    load_sign(b + 1)   # prefetch ahead of this image's evacs
                conv(b, 0, cpsum)
                conv(b, 1, cpsum)
    nc.compile()
    return nc


def _get_nc():
    if "nc" not in _cache:
        _cache["nc"] = _build()
    return _cache["nc"]


def run(inputs, trace=False):
    nc = _get_nc()
    x = np.ascontiguousarray(inputs["x"], dtype=np.float32)
    in_maps = [
        {
            "x": x[c * BL:(c + 1) * BL],
            "weights": np.ascontiguousarray(inputs["weights"], np.float32),
            "RV": np.ascontiguousarray(inputs["RV"], np.float32),
            "alpha": np.ascontiguousarray(inputs["alpha"], np.float32),
        }
        for c in range(NCORES)
    ]
    res = run_bass_kernel_spmd(nc, in_maps, core_ids=list(range(NCORES)),
                               trace=trace)
    out = np.concatenate([r["out"] for r in res.results], axis=0)
    return out, res


def kernel(**inputs) -> np.ndarray:
    out, _ = run(inputs, trace=False)
    return out
